# revision 1
# baseline (speedup 1.0000x reference)
"""DenoiseGCN Trainium2 kernel.

Full-input contract: kernel(**inputs) takes the unsharded inputs from
setup_inputs() and returns the full (512, 2048) float32 output.

Strategy: pure data parallel over 8 NeuronCores (64 samples each, no
collectives). Activations stay resident in SBUF in a feature-major
layout ([features -> partitions, vertices -> free dim]).

Final (v9, 1011.6us HW exec, abs-max rel err 8.8e-3) vs the 1.07ms
baseline (which ran TWO K=256 matmuls per GCN layer, tensor-bound at 97%):
  * stage emission is software-pipelined ACROSS sample groups: group g
    runs stage st at virtual time g*SKEW + st (SKEW=6 over 10 stages),
    so the next group's input/layer-0 head overlaps this group's MLP
    tail - group-sequential emission cost ~7us of PE drain per group.
  * one PSUM pool of four [128,1024] buffers (all 8 banks; the head
    stage's tiny [2,512] tiles allocate from the same pool).
  * each layer now runs ONE K=256 matmul: p = g' @ (W/3) with
    g' = h[v-1] + h[v] + h[v+1] built by two bf16 tensor_tensor adds on
    the vector engine (bf16 SBUF operands hit the DVE 2x_1p mode).
  * the residual +h is added into PSUM either by a bf16 identity-matrix
    matmul on the tensor engine (start=False accumulate) or by a DVE
    tensor_tensor on PSUM, chosen per (sample, layer) to balance engines.
  * body activations/weights are bf16 (abs-max rel err ~1e-2 vs 2e-2
    budget); layer-0, the residual accumulation (fp32 PSUM), and the
    whole MLP head stay fp32r to keep the error down.
  * silu(psum + bias) fused on the scalar engine per m-chunk
    ([128,1024] PSUM reads - fewer, larger activations than baseline).
"""

import numpy as np
import ml_dtypes

import concourse.bacc as bacc
import concourse.mybir as mybir
import concourse.tile as tile
from concourse.bass_utils import run_bass_kernel_spmd

F32 = mybir.dt.float32
F32R = mybir.dt.float32r
BF16 = mybir.dt.bfloat16
AF = mybir.ActivationFunctionType
ALU = mybir.AluOpType

NCORES = 8
B = 512
BPC = B // NCORES  # samples per core
V = 1024           # vertices per sample
HID = 256
TDIM = 128
DATA = 2048
HW = 1026          # haloed row width per feature-tile (1 + 1024 + 1)

def _sin_table():
    half = TDIM // 2
    freqs = np.exp(-np.log(10000.0) * np.arange(half, dtype=np.float64) / (half - 1))
    tt = np.arange(1000, dtype=np.float64)[:, None] * freqs[None, :]
    return np.concatenate([np.sin(tt), np.cos(tt)], axis=1).astype(np.float32)


_SIN_TABLE = _sin_table()

_PROG = None


def _build():
    nc = bacc.Bacc("TRN2", target_bir_lowering=False, debug=False, num_devices=NCORES)

    x = nc.dram_tensor("x", [2 * BPC, V], F32, kind="ExternalInput")
    embT = nc.dram_tensor("embT", [TDIM, BPC], F32, kind="ExternalInput")
    timeW = nc.dram_tensor("timeW", [TDIM, TDIM], F32, kind="ExternalInput")
    timeb = nc.dram_tensor("timeb", [TDIM, 1], F32, kind="ExternalInput")
    w0cr = nc.dram_tensor("w0cr", [4, HID], F32, kind="ExternalInput")
    wsum = nc.dram_tensor("wsum", [TDIM, HID], F32, kind="ExternalInput")
    b0d = nc.dram_tensor("b0", [128, 2], F32, kind="ExternalInput")
    # per-layer (W/3) in bf16, host-rearranged to [128, k*2 x 256] stationary layout
    wld = [nc.dram_tensor(f"w{i}", [128, 2 * HID], BF16, kind="ExternalInput") for i in (1, 2, 3)]
    bld = [nc.dram_tensor(f"b{i}", [128, 2], F32, kind="ExternalInput") for i in (1, 2, 3)]
    eyed = nc.dram_tensor("eye", [128, 128], BF16, kind="ExternalInput")
    hw1 = nc.dram_tensor("hw1", [128, 2 * HID], F32, kind="ExternalInput")
    hb1 = nc.dram_tensor("hb1", [128, 2], F32, kind="ExternalInput")
    hw2 = nc.dram_tensor("hw2", [128, 4], F32, kind="ExternalInput")
    hb2 = nc.dram_tensor("hb2", [2, 1], F32, kind="ExternalInput")
    out = nc.dram_tensor("out", [2 * BPC, V], F32, kind="ExternalOutput")

    with tile.TileContext(nc) as tc:
        with (
            tc.tile_pool(name="const", bufs=1) as pc,
            tc.tile_pool(name="hp", bufs=8) as hp,
            tc.tile_pool(name="h4p", bufs=4) as h4p,
            tc.tile_pool(name="gp", bufs=6) as gp,
            tc.tile_pool(name="hsp", bufs=6) as hsp,
            tc.tile_pool(name="t4p", bufs=6) as t4p,
            tc.tile_pool(name="h5p", bufs=4) as h5p,
            tc.tile_pool(name="op", bufs=4) as op,
            tc.tile_pool(name="ps", bufs=4, space="PSUM") as ps,
        ):
            dma = nc.sync.dma_start
            mm = nc.tensor.matmul
            act = nc.scalar.activation
            tt = nc.vector.tensor_tensor

            def ctile(shape, tag, src_ap=None, dt=F32, eng=None):
                t = pc.tile(shape, dt, tag=tag)
                if src_ap is not None:
                    d = eng.dma_start if eng is not None else dma
                    d(t[:], src_ap.bitcast(dt) if dt is F32R else src_ap)
                return t

            # critical-path constants first (layer-0 chain), bulk after.
            cCoords = ctile([128, V], "ccoords", x[:], dt=F32R)
            cEmb = ctile([TDIM, BPC], "cemb", embT[:], dt=F32R)
            cTW = ctile([TDIM, TDIM], "ctw", timeW[:], dt=F32R)
            ctb = ctile([TDIM, 1], "ctb", timeb[:])
            cWsum = ctile([TDIM, HID], "cwsum", wsum[:], dt=F32R)
            cb0 = ctile([128, 2], "cb0", b0d[:])
            cW0cR = ctile([4, HID], "cw0cr", w0cr[:], dt=F32R)
            cWl = [ctile([128, 2 * HID], f"cw{i}", wld[i][:], dt=BF16, eng=nc.gpsimd)
                   for i in range(3)]
            cBl = [ctile([128, 2], f"cbl{i}", bld[i][:], eng=nc.gpsimd) for i in range(3)]
            cI = ctile([128, 128], "ceye", eyed[:], dt=BF16, eng=nc.gpsimd)
            cHW1 = ctile([128, 2 * HID], "chw1", hw1[:], dt=F32R, eng=nc.gpsimd)
            cHB1 = ctile([128, 2], "chb1", hb1[:], eng=nc.gpsimd)
            cHW2 = ctile([128, 4], "chw2", hw2[:], dt=F32R, eng=nc.gpsimd)
            cHB2c = ctile([2, 1], "chb2c", hb2[:], eng=nc.gpsimd)

            # ---- time embedding MLP: temb = silu(emb @ time_W + time_b) ----
            pt = ps.tile([TDIM, BPC], F32, tag="ps")
            mm(pt[:], (cTW[:]), (cEmb[:]), start=True, stop=True)
            cTemb = ctile([TDIM, BPC], "ctemb", dt=F32R)
            act(cTemb[:], pt[:], AF.Silu, bias=ctb[:])

            # ---- per-sample layer-0 bias columns:
            # cb[:, m*BPC + s] = (temb_s @ (W0[2:]+res0_W[2:]) + b0)[m*128:(m+1)*128]
            cCB = ctile([128, 2 * BPC], "ccb")
            for m in range(2):
                pcb = ps.tile([128, BPC], F32, tag="ps")
                mm(pcb[:], (cWsum[:][:, m * 128:(m + 1) * 128]), (cTemb[:]),
                   start=True, stop=True)
                act(cCB[:][:, m * BPC:(m + 1) * BPC], pcb[:], AF.Identity, bias=cb0[:][:, m:m + 1])

            # ---- batched cycle-agg of coords (raw 3-term sum, no 1/3) ----
            cAggc = ctile([128, V], "caggc", dt=F32R)
            tt(cAggc[:][:, 1:1023], cCoords[:][:, 0:1022], cCoords[:][:, 2:1024], ALU.add)
            tt(cAggc[:][:, 0:1], cCoords[:][:, 1023:1024], cCoords[:][:, 1:2], ALU.add)
            tt(cAggc[:][:, 1023:1024], cCoords[:][:, 1022:1023], cCoords[:][:, 0:1], ALU.add)
            tt(cAggc[:], cAggc[:], cCoords[:], ALU.add)

            st = {}  # per-sample pipeline state

            def halo_fix(s, li, h, h3):
                dma(h3[:, :, 0:1], h3[:, :, 1024:1025])
                dma(h3[:, :, 1025:1026], h3[:, :, 1:2])

            def stage_t4(s):
                t4 = t4p.tile([4, V], F32R, tag="t4", name="t4")
                dma(t4[0:2, :], cAggc[2 * s:2 * s + 2, :])
                dma(t4[2:4, :], cCoords[2 * s:2 * s + 2, :])
                st[s] = {"t4": t4}

            def stage_l0(s):
                # layer 0: h1 = silu(aggc@W0c/3 + coords@res0c + cb_s), bf16 out
                t4 = st[s].pop("t4")
                h = hp.tile([128, 2 * HW], BF16, tag="h", name="h")
                h3 = h[:].rearrange("p (m v) -> p m v", m=2)
                for m in range(2):
                    p = ps.tile([128, 2 * 512], F32, tag="ps", name="pc")
                    for c in range(2):
                        mm(p[:][:, c * 512:(c + 1) * 512],
                           (cW0cR[:][:, m * 128:(m + 1) * 128]),
                           (t4[:][:, c * 512:(c + 1) * 512]),
                           start=True, stop=True)
                    act(h[:][:, m * HW + 1:m * HW + 1 + V],
                        p[:], AF.Silu,
                        bias=cCB[:][:, m * BPC + s:m * BPC + s + 1])
                halo_fix(s, 0, h, h3)
                st[s]["h"] = (h, h3)

            def stage_agg(s, li):
                # aggregation prep: aligned center copy (DMA) + neighbor sum
                h, h3 = st[s]["h"]
                hs = hsp.tile([128, 2 * V], BF16, tag="hs", name="hs")
                hs3 = hs[:].rearrange("p (m v) -> p m v", m=2)
                nc.gpsimd.dma_start(hs3, h3[:, :, 1:1025])
                g = gp.tile([128, 2 * V], BF16, tag="g", name="g")
                g3 = g[:].rearrange("p (m v) -> p m v", m=2)
                nc.vector.tensor_tensor(g3, h3[:, :, 0:1024], h3[:, :, 2:1026], ALU.add)
                st[s]["agg"] = (hs, hs3, g, g3)

            def stage_layer(s, li):
                # h <- silu(cycle_agg(h)@W + h + b), one K=256 matmul per layer
                h, h3 = st[s]["h"]
                cW = cWl[li]
                cB = cBl[li]
                hs, hs3, g, g3 = st[s].pop("agg")
                nc.vector.tensor_tensor(g3, g3, hs3, ALU.add)
                resid_pe = li >= 1
                last = li == 2
                if last:
                    hn = h4p.tile([128, 2 * V], F32R, tag="h4", name="h4")
                    hn3 = None
                else:
                    hn = hp.tile([128, 2 * HW], BF16, tag="h", name="h")
                    hn3 = hn[:].rearrange("p (m v) -> p m v", m=2)
                for m in range(2):
                    p = ps.tile([128, 2 * 512], F32, tag="ps", name="pc")
                    for c in range(2):
                        pcol = p[:][:, c * 512:(c + 1) * 512]
                        for k in range(2):
                            mm(pcol, (cW[:][:, k * HID + m * 128:k * HID + (m + 1) * 128]),
                               (g[:][:, k * V + c * 512:k * V + (c + 1) * 512]),
                               start=(k == 0), stop=(k == 1 and not resid_pe))
                        if resid_pe:
                            mm(pcol, (cI[:]),
                               (hs[:][:, m * V + c * 512:m * V + (c + 1) * 512]),
                               start=False, stop=True)
                    if not resid_pe:
                        nc.vector.tensor_tensor(
                            p[:], p[:], hs3[:, m:m + 1, :], ALU.add)
                    dst = (hn[:][:, m * V:(m + 1) * V] if last
                           else hn[:][:, m * HW + 1:m * HW + 1 + V])
                    act(dst, p[:], AF.Silu, bias=cB[:][:, m:m + 1])
                if not last:
                    halo_fix(s, li + 1, hn, hn3)
                st[s]["h"] = (hn, hn3)

            def stage_m1(s):
                h4, _ = st[s].pop("h")
                h5 = h5p.tile([128, 2 * V], F32R, tag="h5", name="h5")
                for m in range(2):
                    p = ps.tile([128, 2 * 512], F32, tag="ps", name="pc")
                    for c in range(2):
                        pcol = p[:][:, c * 512:(c + 1) * 512]
                        for k in range(2):
                            mm(pcol, (cHW1[:][:, k * HID + m * 128:k * HID + (m + 1) * 128]),
                               (h4[:][:, k * V + c * 512:k * V + (c + 1) * 512]),
                               start=(k == 0), stop=(k == 1))
                    act(h5[:][:, m * V:(m + 1) * V], p[:], AF.Silu, bias=cHB1[:][:, m:m + 1])
                st[s]["h5"] = h5

            def stage_m2(s):
                h5 = st[s].pop("h5")
                osb = op.tile([2, V], F32, tag="osb", name="osb")
                for c in range(2):
                    pm2 = ps.tile([2, 512], F32, tag="ps", name="pm2")
                    mm(pm2[:], (cHW2[:][:, 0:2]), (h5[:][:, c * 512:(c + 1) * 512]),
                       start=True, stop=False)
                    mm(pm2[:], (cHW2[:][:, 2:4]), (h5[:][:, V + c * 512:V + (c + 1) * 512]),
                       start=False, stop=True)
                    nc.vector.tensor_scalar_add(osb[:][:, c * 512:(c + 1) * 512],
                                                pm2[:], cHB2c[:])
                dma(out[2 * s:2 * s + 2, :], osb[:])

            G = 4
            stages = [stage_t4, stage_l0]
            for li in range(3):
                stages.append(lambda s, li=li: stage_agg(s, li))
                stages.append(lambda s, li=li: stage_layer(s, li))
            stages += [stage_m1, stage_m2]
            # software-pipeline across groups: group g runs stage st at
            # time g*SKEW + st, so the next group's t4/l0 stages interleave
            # with this group's m1/m2 tail instead of waiting for it (the
            # group-sequential schedule showed a ~7us PE drain per group).
            NST = len(stages)
            SKEW = 6
            ng = BPC // G
            evs = sorted((g * SKEW + sti, -sti, sti, g)
                         for g in range(ng) for sti in range(NST))
            for _, _, sti, g in evs:
                for s in range(g * G, (g + 1) * G):
                    stages[sti](s)

    nc.compile()
    return nc


def _get_prog():
    global _PROG
    if _PROG is None:
        _PROG = _build()
    return _PROG


def build_in_maps(inputs):
    f = lambda a: np.ascontiguousarray(np.asarray(a, dtype=np.float32))
    tobf = lambda a: np.ascontiguousarray(a.astype(ml_dtypes.bfloat16))
    x = f(inputs["x"])
    t = np.asarray(inputs["t"]).astype(np.int64)
    W0, b0 = f(inputs["W0"]), f(inputs["b0"])
    Ws = [f(inputs[k]) for k in ("W1", "W2", "W3")]
    bs = [f(inputs[k]) for k in ("b1", "b2", "b3")]
    res0_W = f(inputs["res0_W"])
    hW1, hb1 = f(inputs["hW1"]), f(inputs["hb1"])
    hW2, hb2 = f(inputs["hW2"]), f(inputs["hb2"])

    emb = _SIN_TABLE[t]  # (B, TDIM) gather from the constant sinusoid table

    def stat(w):  # [256, N] -> [128, 2*N] stationary layout (k-chunks in free dim)
        n = w.shape[1]
        return w.reshape(2, 128, n).transpose(1, 0, 2).reshape(128, 2 * n)

    def pbias(b):  # [256] -> [128, 2]
        return np.ascontiguousarray(b.reshape(2, 128).T)

    shared = {
        "timeW": f(inputs["time_W"]),
        "timeb": f(inputs["time_b"]).reshape(TDIM, 1),
        "w0cr": np.concatenate([W0[:2] / 3.0, res0_W[:2]], axis=0),
        "wsum": W0[2:] + res0_W[2:],
        "b0": pbias(b0),
        "eye": np.ascontiguousarray(np.eye(128, dtype=ml_dtypes.bfloat16)),
        "hw1": np.ascontiguousarray(stat(hW1)),
        "hb1": pbias(hb1),
        "hw2": np.ascontiguousarray(stat(hW2)),
        "hb2": hb2.reshape(2, 1),
    }
    for i in range(3):
        shared[f"w{i + 1}"] = tobf(stat(Ws[i] / 3.0))
        shared[f"b{i + 1}"] = pbias(bs[i])

    in_maps = []
    for c in range(NCORES):
        sl = slice(c * BPC, (c + 1) * BPC)
        m = dict(shared)
        # (BPC, 2048) -> (BPC, V, 2) -> (BPC, 2, V) -> (2*BPC, V): row 2s+c = x[s, c::2]
        m["x"] = np.ascontiguousarray(
            x[sl].reshape(BPC, V, 2).transpose(0, 2, 1).reshape(2 * BPC, V))
        m["embT"] = np.ascontiguousarray(emb[sl].T)
        in_maps.append(m)
    return in_maps


def kernel(**inputs) -> np.ndarray:
    in_maps = build_in_maps(inputs)
    nc = _get_prog()
    res = run_bass_kernel_spmd(nc, in_maps, list(range(NCORES)))
    outs = []
    for i in range(NCORES):
        o = res.results[i]["out"]  # (2*BPC, V), row 2s+c = out[s, c::2]
        outs.append(o.reshape(BPC, 2, V).transpose(0, 2, 1).reshape(BPC, DATA))
    return np.concatenate(outs, axis=0)


if __name__ == "__main__":
    rng = np.random.default_rng(0)
    demo = {
        "x": rng.standard_normal((B, DATA), dtype=np.float32),
        "t": rng.integers(0, 1000, size=(B,)).astype(np.int32),
        "time_W": rng.standard_normal((TDIM, TDIM), dtype=np.float32) / 11.3,
        "time_b": np.zeros(TDIM, np.float32),
        "W0": rng.standard_normal((130, HID), dtype=np.float32) / 11.4,
        "b0": np.zeros(HID, np.float32),
        "W1": rng.standard_normal((HID, HID), dtype=np.float32) / 16.0,
        "b1": np.zeros(HID, np.float32),
        "W2": rng.standard_normal((HID, HID), dtype=np.float32) / 16.0,
        "b2": np.zeros(HID, np.float32),
        "W3": rng.standard_normal((HID, HID), dtype=np.float32) / 16.0,
        "b3": np.zeros(HID, np.float32),
        "res0_W": rng.standard_normal((130, HID), dtype=np.float32) / 11.4,
        "hW1": rng.standard_normal((HID, HID), dtype=np.float32) / 16.0,
        "hb1": np.zeros(HID, np.float32),
        "hW2": rng.standard_normal((HID, 2), dtype=np.float32) / 16.0,
        "hb2": np.zeros(2, np.float32),
    }
    out = kernel(**demo)
    print("out", out.shape, out.dtype, float(np.abs(out).mean()))

